# revision 1
# baseline (speedup 1.0000x reference)
"""Trainium2 Bass kernel for Conv2dAffine8bit.

Reference computation:
    w_dq = affine-uint8 quantize-dequantize(weight)   (per-tensor scale/zp)
    out  = conv2d(x, w_dq, stride 1, pad 1) + bias    (NCHW, OIHW)
with x [32, 256, 56, 56] f32, weight [256, 256, 3, 3] f32, bias [256] f32.

Strategy (8 NeuronCores, data-parallel over batch, 4 images per core):
  * Host reproduces the quantization math bit-exactly in fp32. The integer
    weights (w_q - zero_point, in [-255, 255]) are EXACT in bf16/f32r, so the
    conv runs against integer weights and `scale` folds into the epilogue
    (conv is linear in w).
  * conv = 9 shifted matmuls (3x3 taps) over a zero-padded [58, 58] image in
    SBUF: out[co, r, :] accumulates lhsT[ci, co] @ x[ci, r+ky, kx:kx+56] into
    PSUM tiles of [128 cout, 8 rows x 56 cols = 448].
  * Two precision modes:
      - "f32r" (default): PE's 12-bit-significand fp32 mode, full bf16-rate
        for free dim >= 256. 9 taps x 2 cin-halves = 18 matmuls per PSUM
        tile; ~1e-4 relative error.
      - "hilo": x split into bf16 hi + lo (x = hi + lo keeps ~16 mantissa
        bits); 36 matmuls per PSUM tile; ~3e-6 relative error, 2x the PE time.
  * Epilogue: ScalarE Identity activation computes psum * scale + bias[co],
    then DMA to the output shard.
"""

import os

import numpy as np
import ml_dtypes

import concourse.tile as tile
from concourse import bacc, mybir
from concourse.bass_utils import run_bass_kernel_spmd

N_CORES = 8
N_IMGS = 32
IMGS_PER_CORE = N_IMGS // N_CORES
C = 256  # in and out channels
H = W = 56
HP = WP = 58  # padded
R = 8  # output rows per PSUM block
N_BLOCKS = H // R  # 7
FREE = R * W  # 448

MODE = os.environ.get("CONV_MODE", "f32r")  # "f32r" | "hilo"

_BF16 = ml_dtypes.bfloat16

_cache: dict = {}


def _build(scale: float, mode: str):
    """Build + compile the per-core Bass program. `scale` is baked as an
    immediate in the epilogue, so cache on it."""
    key = (scale, mode)
    if key in _cache:
        return _cache[key]

    nc = bacc.Bacc()
    dt = mybir.dt
    x_dt = dt.float32r if mode == "f32r" else dt.bfloat16
    w_dt = x_dt
    if mode == "f32r":
        d_xs = [nc.declare_dram_parameter(
            "x0", [IMGS_PER_CORE, C, HP, WP], x_dt, isOutput=False)]
        parts = ("x0",)
    else:
        d_xs = [
            nc.declare_dram_parameter(
                "xhi", [IMGS_PER_CORE, C, HP, WP], x_dt, isOutput=False),
            nc.declare_dram_parameter(
                "xlo", [IMGS_PER_CORE, C, HP, WP], x_dt, isOutput=False),
        ]
        parts = ("xhi", "xlo")
    # wq[ci, ci_half, tap, co_half, co] = (w_q - zp)[co_half*128+co, ci_half*128+ci, tap]
    d_wq = nc.declare_dram_parameter("wq", [128, 2, 9, 2, 128], w_dt, isOutput=False)
    d_bias = nc.declare_dram_parameter("bias", [128, 2], dt.float32, isOutput=False)
    d_y = nc.declare_dram_parameter(
        "y", [IMGS_PER_CORE, C, H, W], dt.float32, isOutput=True)

    with tile.TileContext(nc) as tc:
        with (
            tc.tile_pool(name="wpool", bufs=1) as wpool,
            tc.tile_pool(name="xpool", bufs=3) as xpool,
            tc.tile_pool(name="opool", bufs=4) as opool,
            tc.tile_pool(name="pspool", bufs=4, space="PSUM") as pspool,
        ):
            t_wq = wpool.tile([128, 2, 9, 2, 128], w_dt, tag="wq")
            nc.sync.dma_start(t_wq[:], d_wq[:])
            t_bias = wpool.tile([128, 2], dt.float32, tag="bias")
            nc.sync.dma_start(t_bias[:], d_bias[:])

            n_mm = 18 * len(parts)
            for img in range(IMGS_PER_CORE):
                # x tiles for this image: [128 ci, 58, 58] per (half, part)
                xt = {}
                for ci_half in range(2):
                    for part, src in zip(parts, d_xs):
                        t = xpool.tile([128, HP, WP], x_dt,
                                       tag=f"x_{part}{ci_half}")
                        nc.sync.dma_start(
                            t[:], src[img, ci_half * 128:(ci_half + 1) * 128])
                        xt[(ci_half, part)] = t

                for co_half in range(2):
                    for blk in range(N_BLOCKS):
                        r0 = blk * R
                        ps = pspool.tile([128, FREE], dt.float32, tag="ps")
                        i_mm = 0
                        for ky in (0, -1, 1):
                            for kx in (-1, 0, 1):
                                tap = (ky + 1) * 3 + (kx + 1)
                                for ci_half in range(2):
                                    lhsT = t_wq[:, ci_half, tap, co_half, :]
                                    for part in parts:
                                        rhs = xt[(ci_half, part)][
                                            :, r0 + ky + 1: r0 + ky + 1 + R,
                                            kx + 1: kx + 1 + W]
                                        nc.tensor.matmul(
                                            ps[:], lhsT, rhs,
                                            start=(i_mm == 0),
                                            stop=(i_mm == n_mm - 1))
                                        i_mm += 1
                        ob = opool.tile([128, FREE], dt.float32, tag="ob")
                        nc.scalar.activation(
                            ob[:], ps[:], mybir.ActivationFunctionType.Identity,
                            bias=t_bias[:, co_half:co_half + 1], scale=float(scale))
                        nc.scalar.dma_start(
                            d_y[img, co_half * 128:(co_half + 1) * 128,
                                r0:r0 + R, :],
                            ob[:].rearrange("p (r c) -> p r c", c=W))

    nc.compile()
    _cache[key] = nc
    return nc


def _quantize_weight(weight: np.ndarray):
    """Bit-exact fp32 replica of the reference affine-uint8 quantization.
    Returns (w_int, scale): w_int = w_q - zero_point (integers in [-255, 255],
    exact in bf16) and the per-tensor fp32 scale."""
    w = np.asarray(weight, dtype=np.float32)
    min_val = np.min(w)
    max_val = np.max(w)
    scale = np.float32(np.float32(max_val - min_val) / np.float32(255.0))
    zp = np.round(np.clip(np.float32(255.0) - np.float32(max_val / scale),
                          np.float32(0.0), np.float32(255.0)))
    w_q = np.round(np.clip(zp + w / scale, np.float32(0.0), np.float32(255.0)))
    w_int = (w_q - zp).astype(np.float32)
    return w_int, scale


def kernel(x, weight, bias):
    x = np.asarray(x, dtype=np.float32)
    weight = np.asarray(weight, dtype=np.float32)
    bias = np.asarray(bias, dtype=np.float32)

    w_int, scale = _quantize_weight(weight)

    # lhsT layout [ci, ci_half, tap, co_half, co]
    w_r = w_int.reshape(2, 128, 2, 128, 9)  # [co_half, co, ci_half, ci, tap]
    wq_host = np.ascontiguousarray(np.transpose(w_r, (3, 2, 4, 0, 1)))
    bias_host = np.ascontiguousarray(bias.reshape(2, 128).T)  # [128, 2]

    # pad to [N, C, 58, 58]
    xp = np.zeros((N_IMGS, C, HP, WP), dtype=np.float32)
    xp[:, :, 1:1 + H, 1:1 + W] = x

    if MODE == "f32r":
        x_parts = {"x0": xp}  # raw f32 bits; PE rounds on ingest
        wq_host = wq_host.astype(np.float32)
    else:
        x_hi = xp.astype(_BF16)
        x_lo = (xp - x_hi.astype(np.float32)).astype(_BF16)
        x_parts = {"xhi": x_hi, "xlo": x_lo}
        wq_host = wq_host.astype(_BF16)

    nc = _build(float(scale), MODE)
    in_maps = []
    for c in range(N_CORES):
        sl = slice(c * IMGS_PER_CORE, (c + 1) * IMGS_PER_CORE)
        m = {name: arr[sl] for name, arr in x_parts.items()}
        m["wq"] = wq_host
        m["bias"] = bias_host
        in_maps.append(m)
    res = run_bass_kernel_spmd(nc, in_maps, list(range(N_CORES)))
    return np.concatenate([res.results[c]["y"] for c in range(N_CORES)], axis=0)



# revision 11
# speedup vs baseline: 79654.9979x; 79654.9979x over previous
"""Trainium2 Bass kernel for Conv2dAffine8bit.

Reference computation:
    w_dq = affine-uint8 quantize-dequantize(weight)   (per-tensor scale/zp)
    out  = conv2d(x, w_dq, stride 1, pad 1) + bias    (NCHW, OIHW)
with x [32, 256, 56, 56] f32, weight [256, 256, 3, 3] f32, bias [256] f32.

Strategy (8 NeuronCores, data-parallel over batch, 4 images per core):
  * Host reproduces the quantization math bit-exactly in fp32. The integer
    weights (w_q - zero_point, in [-255, 255]) are EXACT in bf16, so the
    conv runs against integer weights and `scale` folds into the epilogue
    (conv is linear in w).
  * conv = 9 shifted matmuls (3x3 taps) over a zero-padded [58, 58] image in
    SBUF: out[co, r, :] accumulates lhsT[ci, co] @ x[ci, r+ky, kx:kx+56] into
    PSUM tiles of [128 cout, 8 rows x 56 cols = 448].
  * Weight-stationary order: for each (tap, ci_half) the 7 row-block matmuls
    share one stationary weight, amortizing the PE weight-load path.
  * dtype modes (CONV_MODE env):
      - "bf16" (default): x and w in bf16. LDWEIGHTS is split from MATMUL by
        the legalizer and can overlap the previous matmul's streaming via the
        PE's reorder window; FWL doubles the weight-load rate. ~2e-3 rel err.
      - "f32r": PE's 12-bit-significand fp32 mode; every matmul self-loads
        its 4-byte weights. ~1e-4 rel err.
      - "mixed": bf16 weights + f32r activations (self-loading matmul with
        2-byte weights). ~1e-4 rel err.
  * Epilogue: ScalarE Identity activation computes psum * scale + bias[co],
    then DMA to the output shard.
"""

import os

import numpy as np
import ml_dtypes

import concourse.tile as tile
from concourse import bacc, mybir
from concourse.bass_utils import run_bass_kernel_spmd

N_CORES = 8
N_IMGS = 32
IMGS_PER_CORE = N_IMGS // N_CORES
C = 256  # in and out channels
H = W = 56
HP = WP = 58  # padded
R = 8  # output rows per PSUM block
N_BLOCKS = H // R  # 7
FREE = R * W  # 448

MODE = os.environ.get("CONV_MODE", "bf16")  # "bf16" | "f32r" | "mixed"
DEDUP_LDW = os.environ.get("CONV_DEDUP", "0") == "1"

_BF16 = ml_dtypes.bfloat16

_cache: dict = {}


def _mode_dtypes(mode: str):
    dt = mybir.dt
    if mode == "f32r":
        return dt.float32r, dt.float32r
    if mode == "bf16":
        return dt.bfloat16, dt.bfloat16
    if mode == "mixed":
        return dt.float32r, dt.bfloat16
    raise ValueError(mode)


def _emit(nc, scale: float, mode: str, d_x, d_wq, d_bias, d_y,
          reps: int, unroll: int, d_tok=None):
    """Emit the conv program body/bodies inside a TileContext.

    All tiles are allocated statically outside the For_i loop (explicit
    Python round-robin instead of pool rotation) so every access pattern is
    loop-invariant — symbolic APs don't serialize through bass2jax/axon."""
    dt = mybir.dt
    x_dt, w_dt = _mode_dtypes(mode)
    # two x buffer sets (double-buffer across bodies) only fit for 2-byte x
    x_sets = 2 if x_dt == dt.bfloat16 else 1
    N_PS = 8   # all PSUM banks
    N_OB = 8

    with tile.TileContext(nc) as tc:
        with (
            tc.tile_pool(name="wpool", bufs=1) as wpool,
            tc.tile_pool(name="xpool", bufs=1) as xpool,
            tc.tile_pool(name="opool", bufs=1) as opool,
            tc.tile_pool(name="pspool", bufs=1, space="PSUM") as pspool,
        ):
            t_wq = wpool.tile([128, 2, 9, 2, 128], w_dt, tag="wq", name="t_wq")
            nc.sync.dma_start(t_wq[:], d_wq[:])
            t_bias = wpool.tile([128, 2], dt.float32, tag="bias", name="t_bias")
            nc.sync.dma_start(t_bias[:], d_bias[:])

            xts = [
                {(img, h): xpool.tile([128, HP, WP], x_dt,
                                      tag=f"x{s}{img}{h}", name=f"x{s}{img}{h}")
                 for img in range(IMGS_PER_CORE) for h in range(2)}
                for s in range(x_sets)
            ]
            pss_all = [pspool.tile([128, FREE], dt.float32,
                                   tag=f"ps{i}", name=f"ps{i}")
                       for i in range(N_PS)]
            obs_all = [opool.tile([128, FREE], dt.float32,
                                  tag=f"ob{i}", name=f"ob{i}")
                       for i in range(N_OB)]
            counters = {"ps": 0, "ob": 0, "body": 0}

            def body():
                xt = xts[counters["body"] % x_sets]
                counters["body"] += 1
                for img in range(IMGS_PER_CORE):
                    for h in range(2):
                        nc.sync.dma_start(xt[(img, h)][:], d_x[img, h])

                for img in range(IMGS_PER_CORE):
                    for co_half in range(2):
                        pss = []
                        for blk in range(N_BLOCKS):
                            pss.append(pss_all[counters["ps"] % N_PS])
                            counters["ps"] += 1
                        # weight-stationary: 7 row-block matmuls per weight
                        for ti, (ky, kx) in enumerate(
                                (ky, kx) for ky in (0, -1, 1) for kx in (-1, 0, 1)):
                            tap = (ky + 1) * 3 + (kx + 1)
                            for ci_half in range(2):
                                lhsT = t_wq[:, ci_half, tap, co_half, :]
                                for blk in range(N_BLOCKS):
                                    r0 = blk * R
                                    rhs = xt[(img, ci_half)][
                                        :, r0 + ky + 1: r0 + ky + 1 + R,
                                        kx + 1: kx + 1 + W]
                                    nc.tensor.matmul(
                                        pss[blk][:], lhsT, rhs,
                                        start=(ti == 0 and ci_half == 0),
                                        stop=(ti == 8 and ci_half == 1))
                        for blk in range(N_BLOCKS):
                            ob = obs_all[counters["ob"] % N_OB]
                            counters["ob"] += 1
                            nc.scalar.activation(
                                ob[:], pss[blk][:],
                                mybir.ActivationFunctionType.Identity,
                                bias=t_bias[:, co_half:co_half + 1],
                                scale=float(scale))
                            nc.scalar.dma_start(
                                d_y[img, co_half * 128:(co_half + 1) * 128,
                                    blk * R:(blk + 1) * R, :],
                                ob[:].rearrange("p (r c) -> p r c", c=W))

            if reps == 1:
                for _ in range(unroll):
                    body()
            else:
                with tc.For_i(0, reps,
                              hint_engines=(mybir.EngineType.PE,)):
                    for _ in range(unroll):
                        body()
            if d_tok is not None:
                nc.sync.dma_start(d_tok[:], t_bias[:])
            return t_bias


def _dedup_ldweights(nc):
    """Drop InstLdweights whose weight AP is identical to the previous
    weight load on PE with no intervening PE-array-clobbering instruction.
    The tile legalizer emits one LDWEIGHTS per matmul even when consecutive
    matmuls share the stationary operand; the PE array keeps its weights, so
    the reloads are pure overhead. Only sync-free LDWs are dropped (the
    matmuls carry their own semaphore updates)."""
    n_rm = 0
    for blk in nc.main_func.blocks:
        last_sig = None
        keep = []
        for inst in blk.instructions:
            nm = type(inst).__name__
            if nm == "InstLdweights":
                ap = inst.ins[0]
                sig = (str(ap.memref), ap.offset, str(ap.ap))
                si = inst.sync_info
                clean = si is None or (not si.on_wait and not si.on_update)
                if clean and sig == last_sig:
                    n_rm += 1
                    continue
                last_sig = sig
            elif nm == "InstMatmult":
                pass  # streaming doesn't disturb the loaded weights
            elif getattr(inst, "engine", None) == mybir.EngineType.PE:
                last_sig = None  # unknown PE instruction — conservatively reload
            keep.append(inst)
        blk.instructions[:] = keep
    return n_rm


def build_program(scale: float, mode: str, reps: int = 1, unroll: int = 1):
    """The real program: x/wq/bias in, y out."""
    nc = bacc.Bacc()
    dt = mybir.dt
    x_dt, w_dt = _mode_dtypes(mode)
    d_x = nc.declare_dram_parameter(
        "x", [IMGS_PER_CORE, 2, 128, HP, WP], x_dt, isOutput=False)
    # wq[ci, ci_half, tap, co_half, co] = w_int[co_half*128+co, ci_half*128+ci, tap]
    d_wq = nc.declare_dram_parameter("wq", [128, 2, 9, 2, 128], w_dt, isOutput=False)
    d_bias = nc.declare_dram_parameter("bias", [128, 2], dt.float32, isOutput=False)
    d_y = nc.declare_dram_parameter(
        "y", [IMGS_PER_CORE, C, H, W], dt.float32, isOutput=True)
    _emit(nc, scale, mode, d_x, d_wq, d_bias, d_y, reps, unroll)
    if DEDUP_LDW:
        _dedup_ldweights(nc)
    nc.compile()
    return nc


def build_timing_program(scale: float, mode: str, reps: int, unroll: int = 4):
    """Timing-only variant: x and wq live in internal DRAM scratch (garbage
    contents — PE/DMA timing is data-independent), y goes to internal DRAM.
    Host I/O is just bias in / a [128, 2] token out, so the axon-tunnel
    payload is tiny and wall-clock differencing is low-noise."""
    nc = bacc.Bacc()
    dt = mybir.dt
    x_dt, w_dt = _mode_dtypes(mode)
    d_x = nc.dram_tensor("x_scr", [IMGS_PER_CORE, 2, 128, HP, WP], x_dt)
    d_wq = nc.dram_tensor("wq_scr", [128, 2, 9, 2, 128], w_dt)
    d_bias = nc.declare_dram_parameter("bias", [128, 2], dt.float32, isOutput=False)
    d_y = nc.dram_tensor("y_scr", [IMGS_PER_CORE, C, H, W], dt.float32)
    d_tok = nc.declare_dram_parameter("tok", [128, 2], dt.float32, isOutput=True)
    _emit(nc, scale, mode, d_x, d_wq, d_bias, d_y, reps, unroll, d_tok=d_tok)
    if DEDUP_LDW:
        _dedup_ldweights(nc)
    nc.compile()
    return nc


def _build_cached(scale: float, mode: str):
    key = (scale, mode)
    if key not in _cache:
        _cache[key] = build_program(scale, mode)
    return _cache[key]


def _quantize_weight(weight: np.ndarray):
    """Bit-exact fp32 replica of the reference affine-uint8 quantization.
    Returns (w_int, scale): w_int = w_q - zero_point (integers in [-255, 255],
    exact in bf16) and the per-tensor fp32 scale."""
    w = np.asarray(weight, dtype=np.float32)
    min_val = np.min(w)
    max_val = np.max(w)
    scale = np.float32(np.float32(max_val - min_val) / np.float32(255.0))
    zp = np.round(np.clip(np.float32(255.0) - np.float32(max_val / scale),
                          np.float32(0.0), np.float32(255.0)))
    w_q = np.round(np.clip(zp + w / scale, np.float32(0.0), np.float32(255.0)))
    w_int = (w_q - zp).astype(np.float32)
    return w_int, scale


def prepare_inputs(x, weight, bias, mode=None):
    """Host-side prep: quantize weights, lay out lhsT, pad + shard x.
    Returns (in_maps, scale)."""
    mode = mode or MODE
    x = np.asarray(x, dtype=np.float32)
    weight = np.asarray(weight, dtype=np.float32)
    bias = np.asarray(bias, dtype=np.float32)

    w_int, scale = _quantize_weight(weight)

    # lhsT layout [ci, ci_half, tap, co_half, co]
    w_r = w_int.reshape(2, 128, 2, 128, 9)  # [co_half, co, ci_half, ci, tap]
    wq_host = np.ascontiguousarray(np.transpose(w_r, (3, 2, 4, 0, 1)))
    bias_host = np.ascontiguousarray(bias.reshape(2, 128).T)  # [128, 2]

    # pad to [N, 2, 128, 58, 58]
    xp = np.zeros((N_IMGS, 2, 128, HP, WP), dtype=np.float32)
    xp[:, :, :, 1:1 + H, 1:1 + W] = x.reshape(N_IMGS, 2, 128, H, W)

    if mode in ("bf16",):
        xp = xp.astype(_BF16)
    if mode in ("bf16", "mixed"):
        wq_host = wq_host.astype(_BF16)

    in_maps = []
    for c in range(N_CORES):
        sl = slice(c * IMGS_PER_CORE, (c + 1) * IMGS_PER_CORE)
        in_maps.append({"x": xp[sl], "wq": wq_host, "bias": bias_host})
    return in_maps, float(scale)


def kernel(x, weight, bias):
    in_maps, scale = prepare_inputs(x, weight, bias)
    nc = _build_cached(scale, MODE)
    res = run_bass_kernel_spmd(nc, in_maps, list(range(N_CORES)))
    return np.concatenate([res.results[c]["y"] for c in range(N_CORES)], axis=0)


# revision 29
# speedup vs baseline: 95409.6396x; 1.1978x over previous
"""Trainium2 Bass kernel for Conv2dAffine8bit.

Reference computation:
    w_dq = affine-uint8 quantize-dequantize(weight)   (per-tensor scale/zp)
    out  = conv2d(x, w_dq, stride 1, pad 1) + bias    (NCHW, OIHW)
with x [32, 256, 56, 56] f32, weight [256, 256, 3, 3] f32, bias [256] f32.

Strategy (8 NeuronCores, data-parallel over batch, 4 images per core):
  * Host reproduces the quantization math bit-exactly in fp32. The integer
    weights (w_q - zero_point, in [-255, 255]) are EXACT in bf16, so the
    conv runs against integer weights and `scale` folds into the epilogue
    (conv is linear in w).
  * conv = 9 shifted matmuls (3x3 taps) over a zero-padded [58, 58] image in
    SBUF: out[co, r, :] accumulates lhsT[ci, co] @ x[ci, r+ky, kx:kx+56] into
    PSUM tiles of [128 cout, 8 rows x 56 cols = 448].
  * Weight-stationary order: for each (tap, ci_half) the 7 row-block matmuls
    share one stationary weight, amortizing the PE weight-load path.
  * dtype modes (CONV_MODE env):
      - "bf16" (default): x and w in bf16. LDWEIGHTS is split from MATMUL by
        the legalizer and can overlap the previous matmul's streaming via the
        PE's reorder window; FWL doubles the weight-load rate. ~2e-3 rel err.
      - "f32r": PE's 12-bit-significand fp32 mode; every matmul self-loads
        its 4-byte weights. ~1e-4 rel err.
      - "mixed": bf16 weights + f32r activations (self-loading matmul with
        2-byte weights). ~1e-4 rel err.
  * Epilogue: ScalarE Identity activation computes psum * scale + bias[co],
    then DMA to the output shard.
"""

import os

import numpy as np
import ml_dtypes

import concourse.tile as tile
from concourse import bacc, mybir
from concourse.bass_utils import run_bass_kernel_spmd

N_CORES = 8
N_IMGS = 32
IMGS_PER_CORE = N_IMGS // N_CORES
C = 256  # in and out channels
H = W = 56
HP = WP = 58  # padded
R = 8  # output rows per PSUM block
N_BLOCKS = H // R  # 7
FREE = R * W  # 448
CFREE = R * WP  # 464: contiguous-stream variant (2 junk cols per row)
XFLAT = HP * WP + 2  # flat x tile with 1-elem margin on each end

MODE = os.environ.get("CONV_MODE", "bf16")  # "bf16" | "f32r" | "mixed"
DEDUP_LDW = os.environ.get("CONV_DEDUP", "0") == "1"
ORDER = os.environ.get("CONV_ORDER", "tap")  # "tap" | "blk"
CONTIG = os.environ.get("CONV_CONTIG", "1") == "1"

_BF16 = ml_dtypes.bfloat16

_cache: dict = {}


def _mode_dtypes(mode: str):
    dt = mybir.dt
    if mode == "f32r":
        return dt.float32r, dt.float32r
    if mode == "bf16":
        return dt.bfloat16, dt.bfloat16
    if mode == "mixed":
        return dt.float32r, dt.bfloat16
    raise ValueError(mode)


def _emit(nc, scale: float, mode: str, d_x, d_wq, d_bias, d_y,
          reps: int, unroll: int, d_tok=None, order: str = "tap",
          epilogue: bool = True, contig: bool = False):
    """Emit the conv program body/bodies inside a TileContext.

    All tiles are allocated statically outside the For_i loop (explicit
    Python round-robin instead of pool rotation) so every access pattern is
    loop-invariant — symbolic APs don't serialize through bass2jax/axon."""
    dt = mybir.dt
    x_dt, w_dt = _mode_dtypes(mode)
    # two x buffer sets (double-buffer across bodies) only fit for 2-byte x
    x_sets = 2 if x_dt == dt.bfloat16 else 1
    N_PS = 8   # all PSUM banks
    N_OB = 8

    with tile.TileContext(nc) as tc:
        with (
            tc.tile_pool(name="wpool", bufs=1) as wpool,
            tc.tile_pool(name="xpool", bufs=1) as xpool,
            tc.tile_pool(name="opool", bufs=1) as opool,
            tc.tile_pool(name="pspool", bufs=1, space="PSUM") as pspool,
        ):
            t_wq = wpool.tile([128, 2, 9, 2, 128], w_dt, tag="wq", name="t_wq")
            nc.sync.dma_start(t_wq[:], d_wq[:])
            t_bias = wpool.tile([128, 2], dt.float32, tag="bias", name="t_bias")
            nc.sync.dma_start(t_bias[:], d_bias[:])

            x_shape = [128, XFLAT] if contig else [128, HP, WP]
            free = CFREE if contig else FREE
            xts = [
                {(img, h): xpool.tile(x_shape, x_dt,
                                      tag=f"x{s}{img}{h}", name=f"x{s}{img}{h}")
                 for img in range(IMGS_PER_CORE) for h in range(2)}
                for s in range(x_sets)
            ]
            pss_all = [pspool.tile([128, free], dt.float32,
                                   tag=f"ps{i}", name=f"ps{i}")
                       for i in range(N_PS)]
            obs_all = [opool.tile([128, free], dt.float32,
                                  tag=f"ob{i}", name=f"ob{i}")
                       for i in range(N_OB)]
            counters = {"ps": 0, "ob": 0, "body": 0}

            def body():
                xt = xts[counters["body"] % x_sets]
                counters["body"] += 1
                for img in range(IMGS_PER_CORE):
                    for h in range(2):
                        if contig:
                            nc.sync.dma_start(
                                xt[(img, h)][:, 1:1 + HP * WP],
                                d_x[img, h].rearrange("p h w -> p (h w)"))
                        else:
                            nc.sync.dma_start(xt[(img, h)][:], d_x[img, h])

                for img in range(IMGS_PER_CORE):
                    for co_half in range(2):
                        pss = []
                        for blk in range(N_BLOCKS):
                            pss.append(pss_all[counters["ps"] % N_PS])
                            counters["ps"] += 1
                        taps = [(ky, kx) for ky in (0, -1, 1) for kx in (-1, 0, 1)]

                        def mm(blk, ti, ky, kx, ci_half):
                            tap = (ky + 1) * 3 + (kx + 1)
                            lhsT = t_wq[:, ci_half, tap, co_half, :]
                            r0 = blk * R
                            if contig:
                                # stream 8 padded rows flat (464 elems); the
                                # tap shift is a flat offset; 2 wrap columns
                                # per row accumulate junk nobody reads
                                off = (r0 + ky + 1) * WP + kx + 1
                                rhs = xt[(img, ci_half)][:, off:off + CFREE]
                            else:
                                rhs = xt[(img, ci_half)][
                                    :, r0 + ky + 1: r0 + ky + 1 + R,
                                    kx + 1: kx + 1 + W]
                            nc.tensor.matmul(
                                pss[blk][:], lhsT, rhs,
                                start=(ti == 0 and ci_half == 0),
                                stop=(ti == 8 and ci_half == 1))

                        if order == "tap":
                            # weight-stationary: 7 row-block matmuls per weight
                            for ti, (ky, kx) in enumerate(taps):
                                for ci_half in range(2):
                                    for blk in range(N_BLOCKS):
                                        mm(blk, ti, ky, kx, ci_half)
                        else:  # "blk": bank-stationary, 18 matmuls per bank
                            for blk in range(N_BLOCKS):
                                for ti, (ky, kx) in enumerate(taps):
                                    for ci_half in range(2):
                                        mm(blk, ti, ky, kx, ci_half)
                        if not epilogue:
                            continue
                        for blk in range(N_BLOCKS):
                            ob = obs_all[counters["ob"] % N_OB]
                            counters["ob"] += 1
                            nc.scalar.activation(
                                ob[:], pss[blk][:],
                                mybir.ActivationFunctionType.Identity,
                                bias=t_bias[:, co_half:co_half + 1],
                                scale=float(scale))
                            if contig:
                                src = ob[:].rearrange(
                                    "p (r c) -> p r c", c=WP)[:, :, 1:1 + W]
                            else:
                                src = ob[:].rearrange("p (r c) -> p r c", c=W)
                            nc.scalar.dma_start(
                                d_y[img, co_half * 128:(co_half + 1) * 128,
                                    blk * R:(blk + 1) * R, :],
                                src)

            if reps == 1:
                for _ in range(unroll):
                    body()
            else:
                with tc.For_i(0, reps,
                              hint_engines=(mybir.EngineType.PE,)):
                    for _ in range(unroll):
                        body()
            if d_tok is not None:
                nc.sync.dma_start(d_tok[:], t_bias[:])
            return t_bias


def _dedup_ldweights(nc):
    """Drop InstLdweights whose weight AP is identical to the previous
    weight load on PE with no intervening PE-array-clobbering instruction.
    The tile legalizer emits one LDWEIGHTS per matmul even when consecutive
    matmuls share the stationary operand; the PE array keeps its weights, so
    the reloads are pure overhead. Only sync-free LDWs are dropped (the
    matmuls carry their own semaphore updates)."""
    n_rm = 0
    for blk in nc.main_func.blocks:
        last_sig = None
        keep = []
        for inst in blk.instructions:
            nm = type(inst).__name__
            if nm == "InstLdweights":
                ap = inst.ins[0]
                sig = (str(ap.memref), ap.offset, str(ap.ap))
                si = inst.sync_info
                clean = si is None or (not si.on_wait and not si.on_update)
                if clean and sig == last_sig:
                    n_rm += 1
                    continue
                last_sig = sig
            elif nm == "InstMatmult":
                pass  # streaming doesn't disturb the loaded weights
            elif getattr(inst, "engine", None) == mybir.EngineType.PE:
                last_sig = None  # unknown PE instruction — conservatively reload
            keep.append(inst)
        blk.instructions[:] = keep
    return n_rm


def build_program(scale: float, mode: str, reps: int = 1, unroll: int = 1,
                  order: str = None, contig: bool = None):
    """The real program: x/wq/bias in, y out."""
    order = order or ORDER
    contig = CONTIG if contig is None else contig
    nc = bacc.Bacc()
    dt = mybir.dt
    x_dt, w_dt = _mode_dtypes(mode)
    d_x = nc.declare_dram_parameter(
        "x", [IMGS_PER_CORE, 2, 128, HP, WP], x_dt, isOutput=False)
    # wq[ci, ci_half, tap, co_half, co] = w_int[co_half*128+co, ci_half*128+ci, tap]
    d_wq = nc.declare_dram_parameter("wq", [128, 2, 9, 2, 128], w_dt, isOutput=False)
    d_bias = nc.declare_dram_parameter("bias", [128, 2], dt.float32, isOutput=False)
    d_y = nc.declare_dram_parameter(
        "y", [IMGS_PER_CORE, C, H, W], dt.float32, isOutput=True)
    _emit(nc, scale, mode, d_x, d_wq, d_bias, d_y, reps, unroll, order=order,
          contig=contig)
    if DEDUP_LDW:
        _dedup_ldweights(nc)
    nc.compile()
    return nc


def build_timing_program(scale: float, mode: str, reps: int, unroll: int = 4,
                         order: str = None, epilogue: bool = True,
                         contig: bool = None):
    """Timing-only variant: x and wq live in internal DRAM scratch (garbage
    contents — PE/DMA timing is data-independent), y goes to internal DRAM.
    Host I/O is just bias in / a [128, 2] token out, so the axon-tunnel
    payload is tiny and wall-clock differencing is low-noise."""
    nc = bacc.Bacc()
    dt = mybir.dt
    x_dt, w_dt = _mode_dtypes(mode)
    d_x = nc.dram_tensor("x_scr", [IMGS_PER_CORE, 2, 128, HP, WP], x_dt)
    d_wq = nc.dram_tensor("wq_scr", [128, 2, 9, 2, 128], w_dt)
    d_bias = nc.declare_dram_parameter("bias", [128, 2], dt.float32, isOutput=False)
    d_y = nc.dram_tensor("y_scr", [IMGS_PER_CORE, C, H, W], dt.float32)
    d_tok = nc.declare_dram_parameter("tok", [128, 2], dt.float32, isOutput=True)
    _emit(nc, scale, mode, d_x, d_wq, d_bias, d_y, reps, unroll, d_tok=d_tok,
          order=order or ORDER, epilogue=epilogue,
          contig=CONTIG if contig is None else contig)
    if DEDUP_LDW:
        _dedup_ldweights(nc)
    nc.compile()
    return nc


def _build_cached(scale: float, mode: str):
    key = (scale, mode)
    if key not in _cache:
        _cache[key] = build_program(scale, mode)
    return _cache[key]


def _quantize_weight(weight: np.ndarray):
    """Bit-exact fp32 replica of the reference affine-uint8 quantization.
    Returns (w_int, scale): w_int = w_q - zero_point (integers in [-255, 255],
    exact in bf16) and the per-tensor fp32 scale."""
    w = np.asarray(weight, dtype=np.float32)
    min_val = np.min(w)
    max_val = np.max(w)
    scale = np.float32(np.float32(max_val - min_val) / np.float32(255.0))
    zp = np.round(np.clip(np.float32(255.0) - np.float32(max_val / scale),
                          np.float32(0.0), np.float32(255.0)))
    w_q = np.round(np.clip(zp + w / scale, np.float32(0.0), np.float32(255.0)))
    w_int = (w_q - zp).astype(np.float32)
    return w_int, scale


def prepare_inputs(x, weight, bias, mode=None):
    """Host-side prep: quantize weights, lay out lhsT, pad + shard x.
    Returns (in_maps, scale)."""
    mode = mode or MODE
    x = np.asarray(x, dtype=np.float32)
    weight = np.asarray(weight, dtype=np.float32)
    bias = np.asarray(bias, dtype=np.float32)

    w_int, scale = _quantize_weight(weight)

    # lhsT layout [ci, ci_half, tap, co_half, co]
    w_r = w_int.reshape(2, 128, 2, 128, 9)  # [co_half, co, ci_half, ci, tap]
    wq_host = np.ascontiguousarray(np.transpose(w_r, (3, 2, 4, 0, 1)))
    bias_host = np.ascontiguousarray(bias.reshape(2, 128).T)  # [128, 2]

    # pad to [N, 2, 128, 58, 58]
    xp = np.zeros((N_IMGS, 2, 128, HP, WP), dtype=np.float32)
    xp[:, :, :, 1:1 + H, 1:1 + W] = x.reshape(N_IMGS, 2, 128, H, W)

    if mode in ("bf16",):
        xp = xp.astype(_BF16)
    if mode in ("bf16", "mixed"):
        wq_host = wq_host.astype(_BF16)

    in_maps = []
    for c in range(N_CORES):
        sl = slice(c * IMGS_PER_CORE, (c + 1) * IMGS_PER_CORE)
        in_maps.append({"x": xp[sl], "wq": wq_host, "bias": bias_host})
    return in_maps, float(scale)


def kernel(x, weight, bias):
    in_maps, scale = prepare_inputs(x, weight, bias)
    nc = _build_cached(scale, MODE)
    res = run_bass_kernel_spmd(nc, in_maps, list(range(N_CORES)))
    return np.concatenate([res.results[c]["y"] for c in range(N_CORES)], axis=0)
